# revision 12
# baseline (speedup 1.0000x reference)
"""Trainium2 Bass kernel for a local-attention block (MQA, RoPE, causal mask).

Reference computation (B=2, T=2048, WIDTH=2560, 10 q-heads, 1 kv-head,
head_dim=256, window=2048 => mask reduces to causal & same-segment):

    q = x @ wq.T ; k = x @ wk.T ; v = x @ wv.T
    q, k = rope(q), rope(k)
    probs = softmax(q k^T / 16 + mask)
    out = (probs @ v) @ w_final.T + b_final

Sharding: 8 cores = 2 batches x 4 query-blocks of 512 tokens. The single
shared KV head is computed per-core (replicated within a batch). Per-core
host-side token ROTATION by the q-block start makes the device program
identical on every core (SPMD): the core's queries always sit in columns
0:512 of its rotated token axis, and causality is carried by a per-core
0/1 mask input.

Precision strategy: Q/K/V projections run as fp8e4m3 DoubleRow matmuls
(contraction 256/instr at 0.5 cycles/row) with a host-side hi/lo split
of both operands and three bilinear terms (x_hi*w_hi + x_hi*w_lo +
x_lo*w_hi); the dropped x_lo*w_lo term is ~0.1%. Everything downstream
(QK^T, P@V, final projection) runs fp16 (same PE speed as bf16, 8x finer
mantissa).
"""

import sys

import numpy as np

for _p in ("/opt/trn_rl_repo", "/root/.axon_site/_ro/trn_rl_repo"):
    if _p not in sys.path:
        sys.path.insert(0, _p)

import ml_dtypes

FP8 = ml_dtypes.float8_e4m3
FP16 = np.float16

B, T, WIDTH = 2, 2048, 2560
NUM_HEADS, HEAD_DIM = 10, 256
WINDOW = 2048
MAX_WAVELENGTH = 10000.0
QBLK = 512              # query tokens per core
NW = WIDTH // 128       # 20 width stripes
NKP = NW // 2           # 10 DoubleRow contraction pairs
NTT = T // 128          # 16 token tiles
NQS = QBLK // 128       # 4 query sub-tiles
VROW = HEAD_DIM + 1     # v columns + ones column (denominator trick)
# fp8 pre-scales: lift x/w out of the fp8 subnormal floor before hi/lo
# quantization; the product is descaled by DESCALE on eviction.
S_X, S_W = 8.0, 128.0
DESCALE = 1.0 / (S_X * S_W)

_NC_CACHE = {}


def _build_nc():
    """Build the (single, SPMD-uniform) Bass/Tile program."""
    import concourse.bass as bass  # noqa: F401
    import concourse.mybir as mybir
    import concourse.tile as tile
    from concourse import bacc
    from concourse.masks import make_identity

    fp32 = mybir.dt.float32
    fp16 = mybir.dt.float16
    fp8 = mybir.dt.float8e4
    Exp = mybir.ActivationFunctionType.Exp
    Ident = mybir.ActivationFunctionType.Identity
    DR = mybir.MatmulPerfMode.DoubleRow

    nc = bacc.Bacc("TRN2", target_bir_lowering=False, debug=False)

    # ---- DRAM I/O ----
    # x^T stripes, hi/lo fp8, each split into the query block and the rest
    xhq = nc.dram_tensor("xhq", [NW, 128, QBLK], fp8, kind="ExternalInput")
    xhr = nc.dram_tensor("xhr", [NW, 128, T - QBLK], fp8, kind="ExternalInput")
    xlq = nc.dram_tensor("xlq", [NW, 128, QBLK], fp8, kind="ExternalInput")
    xlr = nc.dram_tensor("xlr", [NW, 128, T - QBLK], fp8, kind="ExternalInput")
    wq = nc.dram_tensor("wq", [NW, 2, 128, WIDTH], fp8, kind="ExternalInput")
    wk = nc.dram_tensor("wk", [2, 128, NW * HEAD_DIM], fp8, kind="ExternalInput")
    wv = nc.dram_tensor("wv", [2, 128, NW * HEAD_DIM], fp8, kind="ExternalInput")
    wf = nc.dram_tensor("wf", [NW, 128, WIDTH], fp16, kind="ExternalInput")
    csk = nc.dram_tensor("csk", [64, T], fp16, kind="ExternalInput")
    snk = nc.dram_tensor("snk", [64, T], fp16, kind="ExternalInput")
    msk = nc.dram_tensor("msk", [128, NTT * QBLK], fp16, kind="ExternalInput")
    bia = nc.dram_tensor("bia", [128, NW], fp32, kind="ExternalInput")
    out = nc.dram_tensor("out", [NW, 128, QBLK], fp16, kind="ExternalOutput")

    with tile.TileContext(nc) as tc:
        with (
            tc.tile_pool(name="res", bufs=1) as res,
            tc.tile_pool(name="bigA", bufs=1) as bigA,
            tc.tile_pool(name="bigB", bufs=1) as bigB,
            tc.tile_pool(name="wqs", bufs=3) as wqs,
            tc.tile_pool(name="wfs", bufs=2) as wfs,
            tc.tile_pool(name="ptp", bufs=5) as ptp,
            tc.tile_pool(name="enp", bufs=2) as enp,
            tc.tile_pool(name="tmp", bufs=2) as tmpp,
            tc.tile_pool(name="rcp", bufs=4) as rcpp,
            tc.tile_pool(name="outp", bufs=2) as outp,
            tc.tile_pool(name="pp", bufs=2, space="PSUM") as pp,
            tc.tile_pool(name="stp", bufs=2, space="PSUM") as stp,
            tc.tile_pool(name="op", bufs=4, space="PSUM") as op,
        ):
            # ---- resident SBUF tiles ----
            qtr = res.tile([128, NW, QBLK], fp16, tag="qtr")     # rope'd Q^T
            ktr = res.tile([128, 2, T], fp16, tag="ktr")         # rope'd K^T
            vsb = res.tile([128, NTT * VROW], fp16, tag="vsb")   # V + ones col
            wkr = res.tile([128, 2, NW, HEAD_DIM], fp8, tag="wkr")
            wvr = res.tile([128, 2, NW, HEAD_DIM], fp8, tag="wvr")
            csk_s = res.tile([64, T], fp16, tag="csk")
            snk_s = res.tile([64, T], fp16, tag="snk")
            masks = res.tile([128, NTT * QBLK], fp16, tag="msk")
            bia_s = res.tile([128, NW], fp32, tag="bia")
            ident = res.tile([128, 128], fp16, tag="ident")

            make_identity(nc, ident[:])

            # x^T hi stripes; slot later reused for the attention masks
            x8 = bigA.tile([128, NW, T], fp8, tag="bigA")
            # x^T lo stripes; slot later reused for enc^T
            xl = bigB.tile([128, NW, T], fp8, tag="bigB")

            # SP queue carries only the latency-critical Q-projection
            # stream (xhq + wq stripes); everything else rides the idle
            # Pool-engine SWDGE queue so it never blocks the stream.
            nc.sync.dma_start(out=x8[:, :, 0:QBLK],
                              in_=xhq[:].rearrange("n p m -> p n m"))
            nc.gpsimd.dma_start(out=xl[:, :, 0:QBLK],
                                in_=xlq[:].rearrange("n p m -> p n m"))
            nc.gpsimd.dma_start(out=csk_s[:], in_=csk[:])
            nc.gpsimd.dma_start(out=snk_s[:], in_=snk[:])
            # chunked so each transfer holds the DMA pool < ~2us and the
            # wq stripe stream can interleave
            NCH = 6
            for ch in range(NCH):
                a = (T - QBLK) * ch // NCH
                b = (T - QBLK) * (ch + 1) // NCH
                nc.gpsimd.dma_start(
                    out=x8[:, :, QBLK + a:QBLK + b],
                    in_=xhr[:, :, a:b].rearrange("n p m -> p n m"))
                nc.gpsimd.dma_start(
                    out=xl[:, :, QBLK + a:QBLK + b],
                    in_=xlr[:, :, a:b].rearrange("n p m -> p n m"))
            nc.gpsimd.dma_start(out=wkr[:].rearrange("p s n m -> p s (n m)"),
                                in_=wk[:].rearrange("s p m -> p s m"))
            nc.gpsimd.dma_start(out=wvr[:].rearrange("p s n m -> p s (n m)"),
                                in_=wv[:].rearrange("s p m -> p s m"))
            nc.gpsimd.dma_start(out=bia_s[:], in_=bia[:])
            for ch in range(4):
                a, b = NTT * QBLK * ch // 4, NTT * QBLK * (ch + 1) // 4
                nc.gpsimd.dma_start(out=masks[:, a:b], in_=msk[:, a:b])

            # ones columns of V (denominator of softmax via matmul)
            for t in range(NTT):
                nc.gpsimd.memset(vsb[:, t * VROW + HEAD_DIM: (t + 1) * VROW], 1.0)

            def rope_evict(ps, cs, sn, dst0, dst1):
                """dst0 = ps0*cos - ps1*sin ; dst1 = ps1*cos + ps0*sin.

                ps: [128, n] PSUM fp32; cs/sn: [64, n] SBUF fp32 tables;
                dst0/dst1: fp16 SBUF APs ([0:64],[64:128] of the dest)."""
                n = cs.shape[-1]
                t0 = tmpp.tile([64, QBLK], fp32, tag="t0", name="t0")
                t1 = tmpp.tile([64, QBLK], fp32, tag="t1", name="t1")
                nc.vector.tensor_mul(t0[:, :n], ps[0:64, :], cs)
                nc.vector.tensor_mul(t1[:, :n], ps[64:128, :], sn)
                nc.vector.tensor_sub(dst0, t0[:, :n], t1[:, :n])
                t2 = tmpp.tile([64, QBLK], fp32, tag="t0", name="t2")
                t3 = tmpp.tile([64, QBLK], fp32, tag="t1", name="t3")
                nc.vector.tensor_mul(t2[:, :n], ps[64:128, :], cs)
                nc.vector.tensor_mul(t3[:, :n], ps[0:64, :], sn)
                nc.vector.tensor_add(dst1, t2[:, :n], t3[:, :n])

            def proj3(ps, whi, wlo, xhi_ap, xlo_ap):
                """ps += 3-term hi/lo fp8 DoubleRow product (contraction WIDTH).

                whi/wlo/xhi_ap/xlo_ap: [128, NW, F] fp8 APs (k-stripe dim 2nd).
                """
                terms = ((whi, xhi_ap), (wlo, xhi_ap), (whi, xlo_ap))
                for ti, (wt, xt) in enumerate(terms):
                    for kk in range(NKP):
                        nc.tensor.matmul(
                            ps,
                            lhsT=wt[:, 2 * kk:2 * kk + 2, :],
                            rhs=xt[:, 2 * kk:2 * kk + 2, :],
                            start=(ti == 0 and kk == 0),
                            stop=(ti == 2 and kk == NKP - 1),
                            perf_mode=DR,
                        )

            _ps_pools = [(pp, "pp"), (stp, "st"), (op, "o"), (op, "o"),
                         (stp, "st"), (op, "o"), (pp, "pp"), (op, "o")]

            def proj_ps(i, cols=QBLK):
                pool, tag = _ps_pools[i % len(_ps_pools)]
                return pool.tile([128, cols], fp32, tag=tag, name=f"ps{i}")

            # ---- Q projection -> rope'd Q^T stripes [qdim, QBLK] ----
            # stripe m: qdim rows [128m, 128m+128) = head m//2, half m%2
            for m in range(NW):
                wq_m = wqs.tile([128, 2, NW, 128], fp8, tag="wq")
                nc.sync.dma_start(
                    out=wq_m[:].rearrange("p s n m -> p s (n m)"),
                    in_=wq[m].rearrange("s p m -> p s m"))
                ps = proj_ps(m)
                proj3(ps[:], wq_m[:, 0], wq_m[:, 1],
                      x8[:, :, 0:QBLK], xl[:, :, 0:QBLK])
                if m % 2 == 0:  # rope half of the head dims
                    rope_evict(ps, csk_s[:, 0:QBLK], snk_s[:, 0:QBLK],
                               qtr[0:64, m, :], qtr[64:128, m, :])
                else:           # passthrough half (descale fp8 scaling)
                    nc.scalar.activation(qtr[:, m, :], ps[:], Ident,
                                         scale=DESCALE)

            # ---- K projection -> rope'd K^T [2, 128, T] fp16 ----
            for hh in range(2):
                for g in range(T // QBLK):
                    ps = proj_ps(NW + 4 * hh + g)
                    cols = slice(g * QBLK, (g + 1) * QBLK)
                    proj3(ps[:],
                          wkr[:, 0, :, hh * 128:hh * 128 + 128],
                          wkr[:, 1, :, hh * 128:hh * 128 + 128],
                          x8[:, :, cols], xl[:, :, cols])
                    if hh == 0:
                        rope_evict(ps, csk_s[:, cols], snk_s[:, cols],
                                   ktr[0:64, 0, cols], ktr[64:128, 0, cols])
                    else:
                        nc.scalar.activation(ktr[:, 1, cols], ps[:], Ident,
                                             scale=DESCALE)

            # ---- V projection: x_hi*wv_hi + x_hi*wv_lo + x_lo*wv_hi ----
            for mt in range(NTT):
                ps = proj_ps(NW + 8 + mt, cols=HEAD_DIM)
                toks = slice(mt * 128, (mt + 1) * 128)
                terms = ((x8, 0), (x8, 1), (xl, 0))
                for ti, (xt, s) in enumerate(terms):
                    for kk in range(NKP):
                        nc.tensor.matmul(
                            ps[:],
                            lhsT=xt[:, 2 * kk:2 * kk + 2, toks],
                            rhs=wvr[:, s, 2 * kk:2 * kk + 2, :],
                            start=(ti == 0 and kk == 0),
                            stop=(ti == 2 and kk == NKP - 1),
                            perf_mode=DR,
                        )
                nc.scalar.activation(
                    vsb[:, mt * VROW: mt * VROW + HEAD_DIM], ps[:], Ident,
                    scale=DESCALE)

            # enc^T reuses xl's slot once projections are done
            enct = bigB.tile([128, NW * QBLK], fp16, tag="bigB")

            # ---- attention (S^T layout: k on partitions, q on free dim) ----
            for h in range(NUM_HEADS):
                o_tiles = [op.tile([128, VROW], fp32, tag="o", name=f"o{h}_{i}")
                           for i in range(NQS)]
                for t in range(NTT):
                    st = stp.tile([128, QBLK], fp32, tag="st")
                    nc.tensor.matmul(
                        st[:], lhsT=ktr[:, 0, t * 128:(t + 1) * 128],
                        rhs=qtr[:, 2 * h, :],
                        start=True, stop=False)
                    nc.tensor.matmul(
                        st[:], lhsT=ktr[:, 1, t * 128:(t + 1) * 128],
                        rhs=qtr[:, 2 * h + 1, :],
                        start=False, stop=True)
                    pt = ptp.tile([128, QBLK], fp16, tag="pt")
                    # p = exp(s / sqrt(head_dim)), masked entries -> 0
                    nc.scalar.activation(pt[:], st[:], Exp, scale=0.0625)
                    nc.vector.tensor_mul(
                        pt[:], pt[:], masks[:, t * QBLK:(t + 1) * QBLK])
                    for q4 in range(NQS):
                        nc.tensor.matmul(
                            o_tiles[q4][:],
                            lhsT=pt[:, q4 * 128:(q4 + 1) * 128],
                            rhs=vsb[:, t * VROW:(t + 1) * VROW],
                            start=(t == 0),
                            stop=(t == NTT - 1),
                        )
                for q4 in range(NQS):
                    r = rcpp.tile([128, 1], fp32, tag="r")
                    nc.vector.reciprocal(r[:], o_tiles[q4][:, HEAD_DIM:VROW])
                    en = enp.tile([128, HEAD_DIM], fp16, tag="en")
                    nc.vector.tensor_scalar_mul(
                        en[:], o_tiles[q4][:, 0:HEAD_DIM], r[:])
                    for hh in range(2):
                        tp = pp.tile([128, 128], fp16, tag="pp")
                        nc.tensor.matmul(
                            tp[:], lhsT=en[:, hh * 128:(hh + 1) * 128],
                            rhs=ident[:], is_transpose=True)
                        nc.vector.tensor_copy(
                            enct[:, (2 * h + hh) * QBLK + q4 * 128:
                                 (2 * h + hh) * QBLK + (q4 + 1) * 128],
                            tp[:])

            # ---- final projection: out^T = wf @ enc^T + bias ----
            for m in range(NW):
                wf_m = wfs.tile([128, WIDTH], fp16, tag="wf")
                nc.sync.dma_start(out=wf_m[:], in_=wf[m])
                ps = pp.tile([128, QBLK], fp32, tag="pp")
                for k in range(NW):
                    nc.tensor.matmul(
                        ps[:],
                        lhsT=wf_m[:, k * 128:(k + 1) * 128],
                        rhs=enct[:, k * QBLK:(k + 1) * QBLK],
                        start=(k == 0),
                        stop=(k == NW - 1),
                    )
                osb = outp.tile([128, QBLK], fp16, tag="osb")
                nc.vector.tensor_scalar_add(osb[:], ps[:], bia_s[:, m:m + 1])
                nc.sync.dma_start(out=out[m], in_=osb[:])

    if not nc.is_finalized():
        nc.finalize()  # bacc register allocation — required before walrus compile
    return nc


def get_nc():
    if "nc" not in _NC_CACHE:
        _NC_CACHE["nc"] = _build_nc()
    return _NC_CACHE["nc"]


def _host_prepare(x, segment_pos, wq, wk, wv, w_final, b_final):
    """Build shared + per-core device input arrays."""
    x = np.asarray(x, dtype=np.float32)
    segment_pos = np.asarray(segment_pos)
    wq = np.asarray(wq, dtype=np.float32)
    wk = np.asarray(wk, dtype=np.float32)
    wv = np.asarray(wv, dtype=np.float32)
    w_final = np.asarray(w_final, dtype=np.float32)
    b_final = np.asarray(b_final, dtype=np.float32)

    def hilo(a, s):
        a = a * s
        hi = a.astype(FP8)
        lo = (a - hi.astype(np.float32)).astype(FP8)
        return hi, lo

    def stripes_sq(w):  # [WIDTH, WIDTH] -> [NW,128,WIDTH] w^T stripes (fp32)
        wt = np.ascontiguousarray(w.T)
        return np.ascontiguousarray(
            wt.reshape(NW, 128, NW, 128).transpose(2, 1, 0, 3).reshape(
                NW, 128, WIDTH))

    def skinny(wt):  # [WIDTH, HEAD_DIM] w^T -> [128, NW*HEAD_DIM] (fp32)
        return np.ascontiguousarray(
            wt.reshape(NW, 128, HEAD_DIM).transpose(1, 0, 2).reshape(
                128, NW * HEAD_DIM))

    wq_hi, wq_lo = hilo(stripes_sq(wq), S_W)
    wk_hi, wk_lo = hilo(skinny(np.ascontiguousarray(wk.T)), S_W)
    wv_hi, wv_lo = hilo(skinny(np.ascontiguousarray(wv.T)), S_W)

    shared = {
        "wq": np.ascontiguousarray(np.stack([wq_hi, wq_lo], axis=1)),
        "wk": np.ascontiguousarray(np.stack([wk_hi, wk_lo], axis=0)),
        "wv": np.ascontiguousarray(np.stack([wv_hi, wv_lo], axis=0)),
        "wf": stripes_sq(w_final).astype(FP16),
        "bia": np.ascontiguousarray(b_final.reshape(NW, 128).T).astype(np.float32),
    }

    inv_freq = (
        1.0 / MAX_WAVELENGTH ** (2.0 * np.arange(HEAD_DIM // 4, dtype=np.float32)
                                 / (HEAD_DIM // 2))
    ).astype(np.float32)

    in_maps = []
    for c in range(8):
        b = c // 4
        qs = QBLK * (c % 4)
        perm = (qs + np.arange(T)) % T  # rotated token order

        xrot = x[b][perm]  # [T, WIDTH]
        xT = np.ascontiguousarray(xrot.T)  # [WIDTH, T] fp32
        xT_hi, xT_lo = hilo(xT, S_X)
        xT_hi = xT_hi.reshape(NW, 128, T)
        xT_lo = xT_lo.reshape(NW, 128, T)

        pos = segment_pos[b].astype(np.float32)
        ang = inv_freq[:, None] * pos[perm][None, :]  # [64, T]
        csk_ = (np.cos(ang) * DESCALE).astype(FP16)
        snk_ = (np.sin(ang) * DESCALE).astype(FP16)

        # allow[tq, tk] on original token ids (causal & window & same segment)
        seg = np.cumsum((segment_pos[b] == 0).astype(np.int64))
        tq = qs + np.arange(QBLK)
        tk = perm
        allow = (
            (tk[None, :] <= tq[:, None])
            & (tq[:, None] <= tk[None, :] + WINDOW)
            & (seg[tq][:, None] == seg[tk][None, :])
        )
        # [T(k rot), QBLK] -> [128, NTT*QBLK] tile-major
        mask_kq = np.ascontiguousarray(
            allow.T.reshape(NTT, 128, QBLK).transpose(1, 0, 2).reshape(
                128, NTT * QBLK)).astype(FP16)

        in_maps.append(dict(
            shared,
            xhq=np.ascontiguousarray(xT_hi[:, :, 0:QBLK]),
            xhr=np.ascontiguousarray(xT_hi[:, :, QBLK:T]),
            xlq=np.ascontiguousarray(xT_lo[:, :, 0:QBLK]),
            xlr=np.ascontiguousarray(xT_lo[:, :, QBLK:T]),
            csk=csk_,
            snk=snk_,
            msk=mask_kq,
        ))
    return in_maps


def _assemble(results):
    out = np.empty((B, T, WIDTH), dtype=np.float32)
    for c, res in enumerate(results):
        b, qs = c // 4, QBLK * (c % 4)
        o = np.asarray(res["out"], dtype=np.float32)  # [NW, 128, QBLK]
        out[b, qs:qs + QBLK, :] = o.transpose(2, 0, 1).reshape(QBLK, WIDTH)
    return out


def kernel(x, segment_pos, wq, wk, wv, w_final, b_final):
    from concourse.bass_utils import run_bass_kernel_spmd

    nc = get_nc()
    in_maps = _host_prepare(x, segment_pos, wq, wk, wv, w_final, b_final)
    res = run_bass_kernel_spmd(nc, in_maps, list(range(8)))
    return _assemble(res.results)


# revision 13
# speedup vs baseline: 1.0392x; 1.0392x over previous
"""Trainium2 Bass kernel for a local-attention block (MQA, RoPE, causal mask).

Reference computation (B=2, T=2048, WIDTH=2560, 10 q-heads, 1 kv-head,
head_dim=256, window=2048 => mask reduces to causal & same-segment):

    q = x @ wq.T ; k = x @ wk.T ; v = x @ wv.T
    q, k = rope(q), rope(k)
    probs = softmax(q k^T / 16 + mask)
    out = (probs @ v) @ w_final.T + b_final

Sharding: 8 cores = 2 batches x 4 query-blocks of 512 tokens. The single
shared KV head is computed per-core (replicated within a batch). Per-core
host-side token ROTATION by the q-block start makes the device program
identical on every core (SPMD): the core's queries always sit in columns
0:512 of its rotated token axis, and causality is carried by a per-core
0/1 mask input.

Precision strategy: Q/K/V projections run as fp8e4m3 DoubleRow matmuls
(contraction 256/instr at 0.5 cycles/row) with a host-side hi/lo split
of both operands and three bilinear terms (x_hi*w_hi + x_hi*w_lo +
x_lo*w_hi); the dropped x_lo*w_lo term is ~0.1%. Everything downstream
(QK^T, P@V, final projection) runs fp16 (same PE speed as bf16, 8x finer
mantissa).
"""

import sys

import numpy as np

for _p in ("/opt/trn_rl_repo", "/root/.axon_site/_ro/trn_rl_repo"):
    if _p not in sys.path:
        sys.path.insert(0, _p)

import ml_dtypes

FP8 = ml_dtypes.float8_e4m3
FP16 = np.float16

B, T, WIDTH = 2, 2048, 2560
NUM_HEADS, HEAD_DIM = 10, 256
WINDOW = 2048
MAX_WAVELENGTH = 10000.0
QBLK = 512              # query tokens per core
NW = WIDTH // 128       # 20 width stripes
NKP = NW // 2           # 10 DoubleRow contraction pairs
NTT = T // 128          # 16 token tiles
NQS = QBLK // 128       # 4 query sub-tiles
VROW = HEAD_DIM + 1     # v columns + ones column (denominator trick)
# fp8 pre-scales: lift x/w out of the fp8 subnormal floor before hi/lo
# quantization; the product is descaled by DESCALE on eviction.
S_X, S_W = 8.0, 128.0
DESCALE = 1.0 / (S_X * S_W)

_NC_CACHE = {}


def _build_nc():
    """Build the (single, SPMD-uniform) Bass/Tile program."""
    import concourse.bass as bass  # noqa: F401
    import concourse.mybir as mybir
    import concourse.tile as tile
    from concourse import bacc
    from concourse.masks import make_identity

    fp32 = mybir.dt.float32
    fp16 = mybir.dt.float16
    fp8 = mybir.dt.float8e4
    Exp = mybir.ActivationFunctionType.Exp
    Ident = mybir.ActivationFunctionType.Identity
    DR = mybir.MatmulPerfMode.DoubleRow

    nc = bacc.Bacc("TRN2", target_bir_lowering=False, debug=False)

    # ---- DRAM I/O ----
    # x^T stripes, hi/lo fp8, each split into the query block and the rest
    xhq = nc.dram_tensor("xhq", [NW, 128, QBLK], fp8, kind="ExternalInput")
    xhr = nc.dram_tensor("xhr", [NW, 128, T - QBLK], fp8, kind="ExternalInput")
    xlq = nc.dram_tensor("xlq", [NW, 128, QBLK], fp8, kind="ExternalInput")
    xlr = nc.dram_tensor("xlr", [NW, 128, T - QBLK], fp8, kind="ExternalInput")
    wq = nc.dram_tensor("wq", [NW, 2, 128, WIDTH], fp8, kind="ExternalInput")
    wk = nc.dram_tensor("wk", [2, 128, NW * HEAD_DIM], fp8, kind="ExternalInput")
    wv = nc.dram_tensor("wv", [2, 128, NW * HEAD_DIM], fp8, kind="ExternalInput")
    wf = nc.dram_tensor("wf", [NW, 128, WIDTH], fp16, kind="ExternalInput")
    csk = nc.dram_tensor("csk", [64, T], fp16, kind="ExternalInput")
    snk = nc.dram_tensor("snk", [64, T], fp16, kind="ExternalInput")
    msk = nc.dram_tensor("msk", [128, NTT * QBLK], fp16, kind="ExternalInput")
    bia = nc.dram_tensor("bia", [128, NW], fp32, kind="ExternalInput")
    out = nc.dram_tensor("out", [NW, 128, QBLK], fp16, kind="ExternalOutput")

    with tile.TileContext(nc) as tc:
        with (
            tc.tile_pool(name="res", bufs=1) as res,
            tc.tile_pool(name="bigA", bufs=1) as bigA,
            tc.tile_pool(name="bigB", bufs=1) as bigB,
            tc.tile_pool(name="wqs", bufs=3) as wqs,
            tc.tile_pool(name="wfs", bufs=2) as wfs,
            tc.tile_pool(name="ptp", bufs=5) as ptp,
            tc.tile_pool(name="enp", bufs=2) as enp,
            tc.tile_pool(name="tmp", bufs=2) as tmpp,
            tc.tile_pool(name="rcp", bufs=4) as rcpp,
            tc.tile_pool(name="outp", bufs=2) as outp,
            tc.tile_pool(name="pp", bufs=2, space="PSUM") as pp,
            tc.tile_pool(name="stp", bufs=2, space="PSUM") as stp,
            tc.tile_pool(name="op", bufs=4, space="PSUM") as op,
        ):
            # ---- resident SBUF tiles ----
            qtr = res.tile([128, NW, QBLK], fp16, tag="qtr")     # rope'd Q^T
            ktr = res.tile([128, 2, T], fp16, tag="ktr")         # rope'd K^T
            vsb = res.tile([128, NTT * VROW], fp16, tag="vsb")   # V + ones col
            wkr = res.tile([128, 2, NW, HEAD_DIM], fp8, tag="wkr")
            wvr = res.tile([128, 2, NW, HEAD_DIM], fp8, tag="wvr")
            csk_s = res.tile([64, T], fp16, tag="csk")
            snk_s = res.tile([64, T], fp16, tag="snk")
            masks = res.tile([128, NTT * QBLK], fp16, tag="msk")
            bia_s = res.tile([128, NW], fp32, tag="bia")
            ident = res.tile([128, 128], fp16, tag="ident")

            make_identity(nc, ident[:])

            # x^T hi stripes; slot later reused for the attention masks
            x8 = bigA.tile([128, NW, T], fp8, tag="bigA")
            # x^T lo stripes; slot later reused for enc^T
            xl = bigB.tile([128, NW, T], fp8, tag="bigB")

            # SP queue carries only the latency-critical Q-projection
            # stream (xhq + wq stripes); everything else rides the idle
            # Pool-engine SWDGE queue so it never blocks the stream.
            nc.sync.dma_start(out=x8[:, :, 0:QBLK],
                              in_=xhq[:].rearrange("n p m -> p n m"))
            nc.gpsimd.dma_start(out=xl[:, :, 0:QBLK],
                                in_=xlq[:].rearrange("n p m -> p n m"))
            nc.gpsimd.dma_start(out=csk_s[:], in_=csk[:])
            nc.gpsimd.dma_start(out=snk_s[:], in_=snk[:])
            # chunked (along stripes, keeping DMA lines contiguous) so each
            # transfer holds the DMA pool < ~2.5us and the wq stripe stream
            # can interleave
            for ch in range(5):
                n0, n1 = 4 * ch, 4 * ch + 4
                nc.gpsimd.dma_start(
                    out=x8[:, n0:n1, QBLK:T],
                    in_=xhr[n0:n1].rearrange("n p m -> p n m"))
                nc.gpsimd.dma_start(
                    out=xl[:, n0:n1, QBLK:T],
                    in_=xlr[n0:n1].rearrange("n p m -> p n m"))
            nc.gpsimd.dma_start(out=wkr[:].rearrange("p s n m -> p s (n m)"),
                                in_=wk[:].rearrange("s p m -> p s m"))
            nc.gpsimd.dma_start(out=wvr[:].rearrange("p s n m -> p s (n m)"),
                                in_=wv[:].rearrange("s p m -> p s m"))
            nc.gpsimd.dma_start(out=bia_s[:], in_=bia[:])
            for ch in range(4):
                a, b = NTT * QBLK * ch // 4, NTT * QBLK * (ch + 1) // 4
                nc.gpsimd.dma_start(out=masks[:, a:b], in_=msk[:, a:b])

            # ones columns of V (denominator of softmax via matmul)
            for t in range(NTT):
                nc.gpsimd.memset(vsb[:, t * VROW + HEAD_DIM: (t + 1) * VROW], 1.0)

            def rope_evict(ps, cs, sn, dst0, dst1):
                """dst0 = ps0*cos - ps1*sin ; dst1 = ps1*cos + ps0*sin.

                ps: [128, n] PSUM fp32; cs/sn: [64, n] SBUF fp32 tables;
                dst0/dst1: fp16 SBUF APs ([0:64],[64:128] of the dest)."""
                n = cs.shape[-1]
                t0 = tmpp.tile([64, QBLK], fp32, tag="t0", name="t0")
                t1 = tmpp.tile([64, QBLK], fp32, tag="t1", name="t1")
                nc.vector.tensor_mul(t0[:, :n], ps[0:64, :], cs)
                nc.vector.tensor_mul(t1[:, :n], ps[64:128, :], sn)
                nc.vector.tensor_sub(dst0, t0[:, :n], t1[:, :n])
                t2 = tmpp.tile([64, QBLK], fp32, tag="t0", name="t2")
                t3 = tmpp.tile([64, QBLK], fp32, tag="t1", name="t3")
                nc.vector.tensor_mul(t2[:, :n], ps[64:128, :], cs)
                nc.vector.tensor_mul(t3[:, :n], ps[0:64, :], sn)
                nc.vector.tensor_add(dst1, t2[:, :n], t3[:, :n])

            def proj3(ps, whi, wlo, xhi_ap, xlo_ap):
                """ps += 3-term hi/lo fp8 DoubleRow product (contraction WIDTH).

                whi/wlo/xhi_ap/xlo_ap: [128, NW, F] fp8 APs (k-stripe dim 2nd).
                """
                terms = ((whi, xhi_ap), (wlo, xhi_ap), (whi, xlo_ap))
                for ti, (wt, xt) in enumerate(terms):
                    for kk in range(NKP):
                        nc.tensor.matmul(
                            ps,
                            lhsT=wt[:, 2 * kk:2 * kk + 2, :],
                            rhs=xt[:, 2 * kk:2 * kk + 2, :],
                            start=(ti == 0 and kk == 0),
                            stop=(ti == 2 and kk == NKP - 1),
                            perf_mode=DR,
                        )

            _ps_pools = [(pp, "pp"), (stp, "st"), (op, "o"), (op, "o"),
                         (stp, "st"), (op, "o"), (pp, "pp"), (op, "o")]

            def proj_ps(i, cols=QBLK):
                pool, tag = _ps_pools[i % len(_ps_pools)]
                return pool.tile([128, cols], fp32, tag=tag, name=f"ps{i}")

            # ---- Q projection -> rope'd Q^T stripes [qdim, QBLK] ----
            # stripe m: qdim rows [128m, 128m+128) = head m//2, half m%2
            for m in range(NW):
                wq_m = wqs.tile([128, 2, NW, 128], fp8, tag="wq")
                nc.sync.dma_start(
                    out=wq_m[:].rearrange("p s n m -> p s (n m)"),
                    in_=wq[m].rearrange("s p m -> p s m"))
                ps = proj_ps(m)
                proj3(ps[:], wq_m[:, 0], wq_m[:, 1],
                      x8[:, :, 0:QBLK], xl[:, :, 0:QBLK])
                if m % 2 == 0:  # rope half of the head dims
                    rope_evict(ps, csk_s[:, 0:QBLK], snk_s[:, 0:QBLK],
                               qtr[0:64, m, :], qtr[64:128, m, :])
                else:           # passthrough half (descale fp8 scaling)
                    nc.scalar.activation(qtr[:, m, :], ps[:], Ident,
                                         scale=DESCALE)

            # ---- K projection -> rope'd K^T [2, 128, T] fp16 ----
            for hh in range(2):
                for g in range(T // QBLK):
                    ps = proj_ps(NW + 4 * hh + g)
                    cols = slice(g * QBLK, (g + 1) * QBLK)
                    proj3(ps[:],
                          wkr[:, 0, :, hh * 128:hh * 128 + 128],
                          wkr[:, 1, :, hh * 128:hh * 128 + 128],
                          x8[:, :, cols], xl[:, :, cols])
                    if hh == 0:
                        rope_evict(ps, csk_s[:, cols], snk_s[:, cols],
                                   ktr[0:64, 0, cols], ktr[64:128, 0, cols])
                    else:
                        nc.scalar.activation(ktr[:, 1, cols], ps[:], Ident,
                                             scale=DESCALE)

            # ---- V projection: x_hi*wv_hi + x_hi*wv_lo + x_lo*wv_hi ----
            for mt in range(NTT):
                ps = proj_ps(NW + 8 + mt, cols=HEAD_DIM)
                toks = slice(mt * 128, (mt + 1) * 128)
                terms = ((x8, 0), (x8, 1), (xl, 0))
                for ti, (xt, s) in enumerate(terms):
                    for kk in range(NKP):
                        nc.tensor.matmul(
                            ps[:],
                            lhsT=xt[:, 2 * kk:2 * kk + 2, toks],
                            rhs=wvr[:, s, 2 * kk:2 * kk + 2, :],
                            start=(ti == 0 and kk == 0),
                            stop=(ti == 2 and kk == NKP - 1),
                            perf_mode=DR,
                        )
                nc.scalar.activation(
                    vsb[:, mt * VROW: mt * VROW + HEAD_DIM], ps[:], Ident,
                    scale=DESCALE)

            # enc^T reuses xl's slot once projections are done
            enct = bigB.tile([128, NW * QBLK], fp16, tag="bigB")

            # ---- attention (S^T layout: k on partitions, q on free dim) ----
            for h in range(NUM_HEADS):
                o_tiles = [op.tile([128, VROW], fp32, tag="o", name=f"o{h}_{i}")
                           for i in range(NQS)]
                for t in range(NTT):
                    st = stp.tile([128, QBLK], fp32, tag="st")
                    nc.tensor.matmul(
                        st[:], lhsT=ktr[:, 0, t * 128:(t + 1) * 128],
                        rhs=qtr[:, 2 * h, :],
                        start=True, stop=False)
                    nc.tensor.matmul(
                        st[:], lhsT=ktr[:, 1, t * 128:(t + 1) * 128],
                        rhs=qtr[:, 2 * h + 1, :],
                        start=False, stop=True)
                    pt = ptp.tile([128, QBLK], fp16, tag="pt")
                    # p = exp(s / sqrt(head_dim)), masked entries -> 0
                    nc.scalar.activation(pt[:], st[:], Exp, scale=0.0625)
                    nc.vector.tensor_mul(
                        pt[:], pt[:], masks[:, t * QBLK:(t + 1) * QBLK])
                    for q4 in range(NQS):
                        nc.tensor.matmul(
                            o_tiles[q4][:],
                            lhsT=pt[:, q4 * 128:(q4 + 1) * 128],
                            rhs=vsb[:, t * VROW:(t + 1) * VROW],
                            start=(t == 0),
                            stop=(t == NTT - 1),
                        )
                for q4 in range(NQS):
                    r = rcpp.tile([128, 1], fp32, tag="r")
                    nc.vector.reciprocal(r[:], o_tiles[q4][:, HEAD_DIM:VROW])
                    en = enp.tile([128, HEAD_DIM], fp16, tag="en")
                    nc.vector.tensor_scalar_mul(
                        en[:], o_tiles[q4][:, 0:HEAD_DIM], r[:])
                    for hh in range(2):
                        tp = pp.tile([128, 128], fp16, tag="pp")
                        nc.tensor.matmul(
                            tp[:], lhsT=en[:, hh * 128:(hh + 1) * 128],
                            rhs=ident[:], is_transpose=True)
                        nc.vector.tensor_copy(
                            enct[:, (2 * h + hh) * QBLK + q4 * 128:
                                 (2 * h + hh) * QBLK + (q4 + 1) * 128],
                            tp[:])

            # ---- final projection: out^T = wf @ enc^T + bias ----
            for m in range(NW):
                wf_m = wfs.tile([128, WIDTH], fp16, tag="wf")
                nc.sync.dma_start(out=wf_m[:], in_=wf[m])
                ps = pp.tile([128, QBLK], fp32, tag="pp")
                for k in range(NW):
                    nc.tensor.matmul(
                        ps[:],
                        lhsT=wf_m[:, k * 128:(k + 1) * 128],
                        rhs=enct[:, k * QBLK:(k + 1) * QBLK],
                        start=(k == 0),
                        stop=(k == NW - 1),
                    )
                osb = outp.tile([128, QBLK], fp16, tag="osb")
                nc.vector.tensor_scalar_add(osb[:], ps[:], bia_s[:, m:m + 1])
                nc.sync.dma_start(out=out[m], in_=osb[:])

    if not nc.is_finalized():
        nc.finalize()  # bacc register allocation — required before walrus compile
    return nc


def get_nc():
    if "nc" not in _NC_CACHE:
        _NC_CACHE["nc"] = _build_nc()
    return _NC_CACHE["nc"]


def _host_prepare(x, segment_pos, wq, wk, wv, w_final, b_final):
    """Build shared + per-core device input arrays."""
    x = np.asarray(x, dtype=np.float32)
    segment_pos = np.asarray(segment_pos)
    wq = np.asarray(wq, dtype=np.float32)
    wk = np.asarray(wk, dtype=np.float32)
    wv = np.asarray(wv, dtype=np.float32)
    w_final = np.asarray(w_final, dtype=np.float32)
    b_final = np.asarray(b_final, dtype=np.float32)

    def hilo(a, s):
        a = a * s
        hi = a.astype(FP8)
        lo = (a - hi.astype(np.float32)).astype(FP8)
        return hi, lo

    def stripes_sq(w):  # [WIDTH, WIDTH] -> [NW,128,WIDTH] w^T stripes (fp32)
        wt = np.ascontiguousarray(w.T)
        return np.ascontiguousarray(
            wt.reshape(NW, 128, NW, 128).transpose(2, 1, 0, 3).reshape(
                NW, 128, WIDTH))

    def skinny(wt):  # [WIDTH, HEAD_DIM] w^T -> [128, NW*HEAD_DIM] (fp32)
        return np.ascontiguousarray(
            wt.reshape(NW, 128, HEAD_DIM).transpose(1, 0, 2).reshape(
                128, NW * HEAD_DIM))

    wq_hi, wq_lo = hilo(stripes_sq(wq), S_W)
    wk_hi, wk_lo = hilo(skinny(np.ascontiguousarray(wk.T)), S_W)
    wv_hi, wv_lo = hilo(skinny(np.ascontiguousarray(wv.T)), S_W)

    shared = {
        "wq": np.ascontiguousarray(np.stack([wq_hi, wq_lo], axis=1)),
        "wk": np.ascontiguousarray(np.stack([wk_hi, wk_lo], axis=0)),
        "wv": np.ascontiguousarray(np.stack([wv_hi, wv_lo], axis=0)),
        "wf": stripes_sq(w_final).astype(FP16),
        "bia": np.ascontiguousarray(b_final.reshape(NW, 128).T).astype(np.float32),
    }

    inv_freq = (
        1.0 / MAX_WAVELENGTH ** (2.0 * np.arange(HEAD_DIM // 4, dtype=np.float32)
                                 / (HEAD_DIM // 2))
    ).astype(np.float32)

    in_maps = []
    for c in range(8):
        b = c // 4
        qs = QBLK * (c % 4)
        perm = (qs + np.arange(T)) % T  # rotated token order

        xrot = x[b][perm]  # [T, WIDTH]
        xT = np.ascontiguousarray(xrot.T)  # [WIDTH, T] fp32
        xT_hi, xT_lo = hilo(xT, S_X)
        xT_hi = xT_hi.reshape(NW, 128, T)
        xT_lo = xT_lo.reshape(NW, 128, T)

        pos = segment_pos[b].astype(np.float32)
        ang = inv_freq[:, None] * pos[perm][None, :]  # [64, T]
        csk_ = (np.cos(ang) * DESCALE).astype(FP16)
        snk_ = (np.sin(ang) * DESCALE).astype(FP16)

        # allow[tq, tk] on original token ids (causal & window & same segment)
        seg = np.cumsum((segment_pos[b] == 0).astype(np.int64))
        tq = qs + np.arange(QBLK)
        tk = perm
        allow = (
            (tk[None, :] <= tq[:, None])
            & (tq[:, None] <= tk[None, :] + WINDOW)
            & (seg[tq][:, None] == seg[tk][None, :])
        )
        # [T(k rot), QBLK] -> [128, NTT*QBLK] tile-major
        mask_kq = np.ascontiguousarray(
            allow.T.reshape(NTT, 128, QBLK).transpose(1, 0, 2).reshape(
                128, NTT * QBLK)).astype(FP16)

        in_maps.append(dict(
            shared,
            xhq=np.ascontiguousarray(xT_hi[:, :, 0:QBLK]),
            xhr=np.ascontiguousarray(xT_hi[:, :, QBLK:T]),
            xlq=np.ascontiguousarray(xT_lo[:, :, 0:QBLK]),
            xlr=np.ascontiguousarray(xT_lo[:, :, QBLK:T]),
            csk=csk_,
            snk=snk_,
            msk=mask_kq,
        ))
    return in_maps


def _assemble(results):
    out = np.empty((B, T, WIDTH), dtype=np.float32)
    for c, res in enumerate(results):
        b, qs = c // 4, QBLK * (c % 4)
        o = np.asarray(res["out"], dtype=np.float32)  # [NW, 128, QBLK]
        out[b, qs:qs + QBLK, :] = o.transpose(2, 0, 1).reshape(QBLK, WIDTH)
    return out


def kernel(x, segment_pos, wq, wk, wv, w_final, b_final):
    from concourse.bass_utils import run_bass_kernel_spmd

    nc = get_nc()
    in_maps = _host_prepare(x, segment_pos, wq, wk, wv, w_final, b_final)
    res = run_bass_kernel_spmd(nc, in_maps, list(range(8)))
    return _assemble(res.results)


# revision 14
# speedup vs baseline: 1.0525x; 1.0127x over previous
"""Trainium2 Bass kernel for a local-attention block (MQA, RoPE, causal mask).

Reference computation (B=2, T=2048, WIDTH=2560, 10 q-heads, 1 kv-head,
head_dim=256, window=2048 => mask reduces to causal & same-segment):

    q = x @ wq.T ; k = x @ wk.T ; v = x @ wv.T
    q, k = rope(q), rope(k)
    probs = softmax(q k^T / 16 + mask)
    out = (probs @ v) @ w_final.T + b_final

Sharding: 8 cores = 2 batches x 4 query-blocks of 512 tokens. The single
shared KV head is computed per-core (replicated within a batch). Per-core
host-side token ROTATION by the q-block start makes the device program
identical on every core (SPMD): the core's queries always sit in columns
0:512 of its rotated token axis, and causality is carried by a per-core
0/1 mask input.

Precision strategy: Q/K/V projections run as fp8e4m3 DoubleRow matmuls
(contraction 256/instr at 0.5 cycles/row) with a host-side hi/lo split
of both operands and three bilinear terms (x_hi*w_hi + x_hi*w_lo +
x_lo*w_hi); the dropped x_lo*w_lo term is ~0.1%. Everything downstream
(QK^T, P@V, final projection) runs fp16 (same PE speed as bf16, 8x finer
mantissa).
"""

import sys

import numpy as np

for _p in ("/opt/trn_rl_repo", "/root/.axon_site/_ro/trn_rl_repo"):
    if _p not in sys.path:
        sys.path.insert(0, _p)

import ml_dtypes

FP8 = ml_dtypes.float8_e4m3
FP16 = np.float16

B, T, WIDTH = 2, 2048, 2560
NUM_HEADS, HEAD_DIM = 10, 256
WINDOW = 2048
MAX_WAVELENGTH = 10000.0
QBLK = 512              # query tokens per core
NW = WIDTH // 128       # 20 width stripes
NKP = NW // 2           # 10 DoubleRow contraction pairs
NTT = T // 128          # 16 token tiles
NQS = QBLK // 128       # 4 query sub-tiles
VROW = HEAD_DIM + 1     # v columns + ones column (denominator trick)
# fp8 pre-scales: lift x/w out of the fp8 subnormal floor before hi/lo
# quantization; the product is descaled by DESCALE on eviction.
S_X, S_W = 8.0, 128.0
DESCALE = 1.0 / (S_X * S_W)

_NC_CACHE = {}


def _build_nc():
    """Build the (single, SPMD-uniform) Bass/Tile program."""
    import concourse.bass as bass  # noqa: F401
    import concourse.mybir as mybir
    import concourse.tile as tile
    from concourse import bacc
    from concourse.masks import make_identity

    fp32 = mybir.dt.float32
    fp16 = mybir.dt.float16
    fp8 = mybir.dt.float8e4
    Exp = mybir.ActivationFunctionType.Exp
    Ident = mybir.ActivationFunctionType.Identity
    DR = mybir.MatmulPerfMode.DoubleRow

    nc = bacc.Bacc("TRN2", target_bir_lowering=False, debug=False)

    # ---- DRAM I/O ----
    # x^T stripes, hi/lo fp8, each split into the query block and the rest
    xhq = nc.dram_tensor("xhq", [NW, 128, QBLK], fp8, kind="ExternalInput")
    xhr = nc.dram_tensor("xhr", [NW, 128, T - QBLK], fp8, kind="ExternalInput")
    xlq = nc.dram_tensor("xlq", [NW, 128, QBLK], fp8, kind="ExternalInput")
    xlr = nc.dram_tensor("xlr", [NW, 128, T - QBLK], fp8, kind="ExternalInput")
    wq = nc.dram_tensor("wq", [NW, 2, 128, WIDTH], fp8, kind="ExternalInput")
    wk = nc.dram_tensor("wk", [2, 128, NW * HEAD_DIM], fp8, kind="ExternalInput")
    wv = nc.dram_tensor("wv", [2, 128, NW * HEAD_DIM], fp8, kind="ExternalInput")
    wf = nc.dram_tensor("wf", [NW, 128, WIDTH], fp16, kind="ExternalInput")
    csk = nc.dram_tensor("csk", [64, T], fp16, kind="ExternalInput")
    snk = nc.dram_tensor("snk", [64, T], fp16, kind="ExternalInput")
    msk = nc.dram_tensor("msk", [128, NTT * QBLK], fp16, kind="ExternalInput")
    bia = nc.dram_tensor("bia", [128, NW], fp32, kind="ExternalInput")
    out = nc.dram_tensor("out", [NW, 128, QBLK], fp16, kind="ExternalOutput")

    with tile.TileContext(nc) as tc:
        with (
            tc.tile_pool(name="res", bufs=1) as res,
            tc.tile_pool(name="bigA", bufs=1) as bigA,
            tc.tile_pool(name="bigB", bufs=1) as bigB,
            tc.tile_pool(name="wqs", bufs=3) as wqs,
            tc.tile_pool(name="wfs", bufs=2) as wfs,
            tc.tile_pool(name="ptp", bufs=5) as ptp,
            tc.tile_pool(name="enp", bufs=2) as enp,
            tc.tile_pool(name="tmp", bufs=2) as tmpp,
            tc.tile_pool(name="rcp", bufs=4) as rcpp,
            tc.tile_pool(name="outp", bufs=2) as outp,
            tc.tile_pool(name="pp", bufs=2, space="PSUM") as pp,
            tc.tile_pool(name="stp", bufs=2, space="PSUM") as stp,
            tc.tile_pool(name="op", bufs=4, space="PSUM") as op,
        ):
            # ---- resident SBUF tiles ----
            qtr = res.tile([128, NW, QBLK], fp16, tag="qtr")     # rope'd Q^T
            ktr = res.tile([128, 2, T], fp16, tag="ktr")         # rope'd K^T
            vsb = res.tile([128, NTT * VROW], fp16, tag="vsb")   # V + ones col
            wkr = res.tile([128, 2, NW, HEAD_DIM], fp8, tag="wkr")
            wvr = res.tile([128, 2, NW, HEAD_DIM], fp8, tag="wvr")
            csk_s = res.tile([64, T], fp16, tag="csk")
            snk_s = res.tile([64, T], fp16, tag="snk")
            masks = res.tile([128, NTT * QBLK], fp16, tag="msk")
            bia_s = res.tile([128, NW], fp32, tag="bia")
            ident = res.tile([128, 128], fp16, tag="ident")

            make_identity(nc, ident[:])

            # x^T hi stripes; slot later reused for the attention masks
            x8 = bigA.tile([128, NW, T], fp8, tag="bigA")
            # x^T lo stripes; slot later reused for enc^T
            xl = bigB.tile([128, NW, T], fp8, tag="bigB")

            # SP queue carries only the latency-critical Q-projection
            # stream (xhq + wq stripes); everything else rides the idle
            # Pool-engine SWDGE queue so it never blocks the stream.
            nc.sync.dma_start(out=x8[:, 0:4, 0:QBLK],
                              in_=xhq[0:4].rearrange("n p m -> p n m"))
            nc.sync.dma_start(out=x8[:, 4:NW, 0:QBLK],
                              in_=xhq[4:NW].rearrange("n p m -> p n m"))
            nc.gpsimd.dma_start(out=xl[:, :, 0:QBLK],
                                in_=xlq[:].rearrange("n p m -> p n m"))
            nc.gpsimd.dma_start(out=csk_s[:], in_=csk[:])
            nc.gpsimd.dma_start(out=snk_s[:], in_=snk[:])
            # bulk loads, column-chunked (512-byte contiguous lines) in the
            # order the K/V projections will consume them; each transfer
            # holds the DMA pool < ~4us so the wq stripe stream interleaves
            for ch in range(3):
                a, b = QBLK * ch, QBLK * (ch + 1)
                nc.gpsimd.dma_start(
                    out=x8[:, :, QBLK + a:QBLK + b],
                    in_=xhr[:, :, a:b].rearrange("n p m -> p n m"))
            nc.gpsimd.dma_start(out=wkr[:].rearrange("p s n m -> p s (n m)"),
                                in_=wk[:].rearrange("s p m -> p s m"))
            for ch in range(3):
                a, b = QBLK * ch, QBLK * (ch + 1)
                nc.gpsimd.dma_start(
                    out=xl[:, :, QBLK + a:QBLK + b],
                    in_=xlr[:, :, a:b].rearrange("n p m -> p n m"))
            nc.gpsimd.dma_start(out=wvr[:].rearrange("p s n m -> p s (n m)"),
                                in_=wv[:].rearrange("s p m -> p s m"))
            nc.gpsimd.dma_start(out=bia_s[:], in_=bia[:])
            for ch in range(4):
                a, b = NTT * QBLK * ch // 4, NTT * QBLK * (ch + 1) // 4
                nc.gpsimd.dma_start(out=masks[:, a:b], in_=msk[:, a:b])

            # ones columns of V (denominator of softmax via matmul)
            for t in range(NTT):
                nc.gpsimd.memset(vsb[:, t * VROW + HEAD_DIM: (t + 1) * VROW], 1.0)

            def rope_evict(ps, cs, sn, dst0, dst1):
                """dst0 = ps0*cos - ps1*sin ; dst1 = ps1*cos + ps0*sin.

                ps: [128, n] PSUM fp32; cs/sn: [64, n] SBUF fp32 tables;
                dst0/dst1: fp16 SBUF APs ([0:64],[64:128] of the dest)."""
                n = cs.shape[-1]
                t0 = tmpp.tile([64, QBLK], fp32, tag="t0", name="t0")
                t1 = tmpp.tile([64, QBLK], fp32, tag="t1", name="t1")
                nc.vector.tensor_mul(t0[:, :n], ps[0:64, :], cs)
                nc.vector.tensor_mul(t1[:, :n], ps[64:128, :], sn)
                nc.vector.tensor_sub(dst0, t0[:, :n], t1[:, :n])
                t2 = tmpp.tile([64, QBLK], fp32, tag="t0", name="t2")
                t3 = tmpp.tile([64, QBLK], fp32, tag="t1", name="t3")
                nc.vector.tensor_mul(t2[:, :n], ps[64:128, :], cs)
                nc.vector.tensor_mul(t3[:, :n], ps[0:64, :], sn)
                nc.vector.tensor_add(dst1, t2[:, :n], t3[:, :n])

            def proj3(ps, whi, wlo, xhi_ap, xlo_ap):
                """ps += 3-term hi/lo fp8 DoubleRow product (contraction WIDTH).

                whi/wlo/xhi_ap/xlo_ap: [128, NW, F] fp8 APs (k-stripe dim 2nd).
                """
                terms = ((whi, xhi_ap), (wlo, xhi_ap), (whi, xlo_ap))
                for ti, (wt, xt) in enumerate(terms):
                    for kk in range(NKP):
                        nc.tensor.matmul(
                            ps,
                            lhsT=wt[:, 2 * kk:2 * kk + 2, :],
                            rhs=xt[:, 2 * kk:2 * kk + 2, :],
                            start=(ti == 0 and kk == 0),
                            stop=(ti == 2 and kk == NKP - 1),
                            perf_mode=DR,
                        )

            _ps_pools = [(pp, "pp"), (stp, "st"), (op, "o"), (op, "o"),
                         (stp, "st"), (op, "o"), (pp, "pp"), (op, "o")]

            def proj_ps(i, cols=QBLK):
                pool, tag = _ps_pools[i % len(_ps_pools)]
                return pool.tile([128, cols], fp32, tag=tag, name=f"ps{i}")

            # ---- Q projection -> rope'd Q^T stripes [qdim, QBLK] ----
            # stripe m: qdim rows [128m, 128m+128) = head m//2, half m%2
            for m in range(NW):
                wq_m = wqs.tile([128, 2, NW, 128], fp8, tag="wq")
                for s in range(2):  # hi first: the first matmuls need only hi
                    nc.sync.dma_start(
                        out=wq_m[:, s].rearrange("p n m -> p (n m)"),
                        in_=wq[m, s])
                ps = proj_ps(m)
                proj3(ps[:], wq_m[:, 0], wq_m[:, 1],
                      x8[:, :, 0:QBLK], xl[:, :, 0:QBLK])
                if m % 2 == 0:  # rope half of the head dims
                    rope_evict(ps, csk_s[:, 0:QBLK], snk_s[:, 0:QBLK],
                               qtr[0:64, m, :], qtr[64:128, m, :])
                else:           # passthrough half (descale fp8 scaling)
                    nc.scalar.activation(qtr[:, m, :], ps[:], Ident,
                                         scale=DESCALE)

            # ---- K projection -> rope'd K^T [2, 128, T] fp16 ----
            for hh in range(2):
                for g in range(T // QBLK):
                    ps = proj_ps(NW + 4 * hh + g)
                    cols = slice(g * QBLK, (g + 1) * QBLK)
                    proj3(ps[:],
                          wkr[:, 0, :, hh * 128:hh * 128 + 128],
                          wkr[:, 1, :, hh * 128:hh * 128 + 128],
                          x8[:, :, cols], xl[:, :, cols])
                    if hh == 0:
                        rope_evict(ps, csk_s[:, cols], snk_s[:, cols],
                                   ktr[0:64, 0, cols], ktr[64:128, 0, cols])
                    else:
                        nc.scalar.activation(ktr[:, 1, cols], ps[:], Ident,
                                             scale=DESCALE)

            # ---- V projection: x_hi*wv_hi + x_hi*wv_lo + x_lo*wv_hi ----
            for mt in range(NTT):
                ps = proj_ps(NW + 8 + mt, cols=HEAD_DIM)
                toks = slice(mt * 128, (mt + 1) * 128)
                terms = ((x8, 0), (x8, 1), (xl, 0))
                for ti, (xt, s) in enumerate(terms):
                    for kk in range(NKP):
                        nc.tensor.matmul(
                            ps[:],
                            lhsT=xt[:, 2 * kk:2 * kk + 2, toks],
                            rhs=wvr[:, s, 2 * kk:2 * kk + 2, :],
                            start=(ti == 0 and kk == 0),
                            stop=(ti == 2 and kk == NKP - 1),
                            perf_mode=DR,
                        )
                nc.scalar.activation(
                    vsb[:, mt * VROW: mt * VROW + HEAD_DIM], ps[:], Ident,
                    scale=DESCALE)

            # enc^T reuses xl's slot once projections are done
            enct = bigB.tile([128, NW * QBLK], fp16, tag="bigB")

            # ---- attention (S^T layout: k on partitions, q on free dim) ----
            for h in range(NUM_HEADS):
                o_tiles = [op.tile([128, VROW], fp32, tag="o", name=f"o{h}_{i}")
                           for i in range(NQS)]
                for t in range(NTT):
                    st = stp.tile([128, QBLK], fp32, tag="st")
                    nc.tensor.matmul(
                        st[:], lhsT=ktr[:, 0, t * 128:(t + 1) * 128],
                        rhs=qtr[:, 2 * h, :],
                        start=True, stop=False)
                    nc.tensor.matmul(
                        st[:], lhsT=ktr[:, 1, t * 128:(t + 1) * 128],
                        rhs=qtr[:, 2 * h + 1, :],
                        start=False, stop=True)
                    pt = ptp.tile([128, QBLK], fp16, tag="pt")
                    # p = exp(s / sqrt(head_dim)), masked entries -> 0
                    nc.scalar.activation(pt[:], st[:], Exp, scale=0.0625)
                    nc.vector.tensor_mul(
                        pt[:], pt[:], masks[:, t * QBLK:(t + 1) * QBLK])
                    for q4 in range(NQS):
                        nc.tensor.matmul(
                            o_tiles[q4][:],
                            lhsT=pt[:, q4 * 128:(q4 + 1) * 128],
                            rhs=vsb[:, t * VROW:(t + 1) * VROW],
                            start=(t == 0),
                            stop=(t == NTT - 1),
                        )
                for q4 in range(NQS):
                    r = rcpp.tile([128, 1], fp32, tag="r")
                    nc.vector.reciprocal(r[:], o_tiles[q4][:, HEAD_DIM:VROW])
                    en = enp.tile([128, HEAD_DIM], fp16, tag="en")
                    nc.vector.tensor_scalar_mul(
                        en[:], o_tiles[q4][:, 0:HEAD_DIM], r[:])
                    for hh in range(2):
                        tp = pp.tile([128, 128], fp16, tag="pp")
                        nc.tensor.matmul(
                            tp[:], lhsT=en[:, hh * 128:(hh + 1) * 128],
                            rhs=ident[:], is_transpose=True)
                        nc.vector.tensor_copy(
                            enct[:, (2 * h + hh) * QBLK + q4 * 128:
                                 (2 * h + hh) * QBLK + (q4 + 1) * 128],
                            tp[:])

            # ---- final projection: out^T = wf @ enc^T + bias ----
            for m in range(NW):
                wf_m = wfs.tile([128, WIDTH], fp16, tag="wf")
                nc.sync.dma_start(out=wf_m[:], in_=wf[m])
                ps = pp.tile([128, QBLK], fp32, tag="pp")
                for k in range(NW):
                    nc.tensor.matmul(
                        ps[:],
                        lhsT=wf_m[:, k * 128:(k + 1) * 128],
                        rhs=enct[:, k * QBLK:(k + 1) * QBLK],
                        start=(k == 0),
                        stop=(k == NW - 1),
                    )
                osb = outp.tile([128, QBLK], fp16, tag="osb")
                nc.vector.tensor_scalar_add(osb[:], ps[:], bia_s[:, m:m + 1])
                nc.sync.dma_start(out=out[m], in_=osb[:])

    if not nc.is_finalized():
        nc.finalize()  # bacc register allocation — required before walrus compile
    return nc


def get_nc():
    if "nc" not in _NC_CACHE:
        _NC_CACHE["nc"] = _build_nc()
    return _NC_CACHE["nc"]


def _host_prepare(x, segment_pos, wq, wk, wv, w_final, b_final):
    """Build shared + per-core device input arrays."""
    x = np.asarray(x, dtype=np.float32)
    segment_pos = np.asarray(segment_pos)
    wq = np.asarray(wq, dtype=np.float32)
    wk = np.asarray(wk, dtype=np.float32)
    wv = np.asarray(wv, dtype=np.float32)
    w_final = np.asarray(w_final, dtype=np.float32)
    b_final = np.asarray(b_final, dtype=np.float32)

    def hilo(a, s):
        a = a * s
        hi = a.astype(FP8)
        lo = (a - hi.astype(np.float32)).astype(FP8)
        return hi, lo

    def stripes_sq(w):  # [WIDTH, WIDTH] -> [NW,128,WIDTH] w^T stripes (fp32)
        wt = np.ascontiguousarray(w.T)
        return np.ascontiguousarray(
            wt.reshape(NW, 128, NW, 128).transpose(2, 1, 0, 3).reshape(
                NW, 128, WIDTH))

    def skinny(wt):  # [WIDTH, HEAD_DIM] w^T -> [128, NW*HEAD_DIM] (fp32)
        return np.ascontiguousarray(
            wt.reshape(NW, 128, HEAD_DIM).transpose(1, 0, 2).reshape(
                128, NW * HEAD_DIM))

    wq_hi, wq_lo = hilo(stripes_sq(wq), S_W)
    wk_hi, wk_lo = hilo(skinny(np.ascontiguousarray(wk.T)), S_W)
    wv_hi, wv_lo = hilo(skinny(np.ascontiguousarray(wv.T)), S_W)

    shared = {
        "wq": np.ascontiguousarray(np.stack([wq_hi, wq_lo], axis=1)),
        "wk": np.ascontiguousarray(np.stack([wk_hi, wk_lo], axis=0)),
        "wv": np.ascontiguousarray(np.stack([wv_hi, wv_lo], axis=0)),
        "wf": stripes_sq(w_final).astype(FP16),
        "bia": np.ascontiguousarray(b_final.reshape(NW, 128).T).astype(np.float32),
    }

    inv_freq = (
        1.0 / MAX_WAVELENGTH ** (2.0 * np.arange(HEAD_DIM // 4, dtype=np.float32)
                                 / (HEAD_DIM // 2))
    ).astype(np.float32)

    in_maps = []
    for c in range(8):
        b = c // 4
        qs = QBLK * (c % 4)
        perm = (qs + np.arange(T)) % T  # rotated token order

        xrot = x[b][perm]  # [T, WIDTH]
        xT = np.ascontiguousarray(xrot.T)  # [WIDTH, T] fp32
        xT_hi, xT_lo = hilo(xT, S_X)
        xT_hi = xT_hi.reshape(NW, 128, T)
        xT_lo = xT_lo.reshape(NW, 128, T)

        pos = segment_pos[b].astype(np.float32)
        ang = inv_freq[:, None] * pos[perm][None, :]  # [64, T]
        csk_ = (np.cos(ang) * DESCALE).astype(FP16)
        snk_ = (np.sin(ang) * DESCALE).astype(FP16)

        # allow[tq, tk] on original token ids (causal & window & same segment)
        seg = np.cumsum((segment_pos[b] == 0).astype(np.int64))
        tq = qs + np.arange(QBLK)
        tk = perm
        allow = (
            (tk[None, :] <= tq[:, None])
            & (tq[:, None] <= tk[None, :] + WINDOW)
            & (seg[tq][:, None] == seg[tk][None, :])
        )
        # [T(k rot), QBLK] -> [128, NTT*QBLK] tile-major
        mask_kq = np.ascontiguousarray(
            allow.T.reshape(NTT, 128, QBLK).transpose(1, 0, 2).reshape(
                128, NTT * QBLK)).astype(FP16)

        in_maps.append(dict(
            shared,
            xhq=np.ascontiguousarray(xT_hi[:, :, 0:QBLK]),
            xhr=np.ascontiguousarray(xT_hi[:, :, QBLK:T]),
            xlq=np.ascontiguousarray(xT_lo[:, :, 0:QBLK]),
            xlr=np.ascontiguousarray(xT_lo[:, :, QBLK:T]),
            csk=csk_,
            snk=snk_,
            msk=mask_kq,
        ))
    return in_maps


def _assemble(results):
    out = np.empty((B, T, WIDTH), dtype=np.float32)
    for c, res in enumerate(results):
        b, qs = c // 4, QBLK * (c % 4)
        o = np.asarray(res["out"], dtype=np.float32)  # [NW, 128, QBLK]
        out[b, qs:qs + QBLK, :] = o.transpose(2, 0, 1).reshape(QBLK, WIDTH)
    return out


def kernel(x, segment_pos, wq, wk, wv, w_final, b_final):
    from concourse.bass_utils import run_bass_kernel_spmd

    nc = get_nc()
    in_maps = _host_prepare(x, segment_pos, wq, wk, wv, w_final, b_final)
    res = run_bass_kernel_spmd(nc, in_maps, list(range(8)))
    return _assemble(res.results)
